# revision 1
# baseline (speedup 1.0000x reference)
"""Distributed matvec kernel for nn_CubicalModel_ISM.

Computes Xp = I @ p, Yp = J @ p with I, J: [784, 50000], p: [50000], then
gathers tiny [50, 2] persistence diagrams from the 28x28 reshapes.

Strategy (8 NeuronCores):
  - Shard the contraction dim P=50000 column-wise across 8 cores
    (6400 = 50*128 per core, zero-padded at the tail).
  - Host-side: transpose each shard to [K, 784], split fp32 into bf16
    hi + bf16 lo planes (same total bytes as fp32, so the memory
    roofline is unchanged, but the PE runs at bf16 rate instead of the
    4x-slower fp32 mode), and pack pairs of 128-row k-subtiles
    side-by-side so each DMA moves a fully contiguous [128 x 3136B]
    block (802 KB). p is split the same way; products
    hi*hi + hi*lo + lo*hi are accumulated in fp32 PSUM, recovering
    fp32-level precision (dropped lo*lo term is ~2^-18 relative).
  - Raw Bass (no Tile): this walrus build supports only ONE sync-wait
    per DMA instruction. Each DMA carries exactly one embedded wait --
    on its own round-robin lane's predecessor -- which strictly orders
    every lane's semaphore updates (race-free counts). All other waits
    are standalone engine wait_ge ops.
  - Host: sum the 8 cores' partials (the "all-reduce"), reshape, gather.
"""

import numpy as np
import ml_dtypes

import concourse.bass as bass
import concourse.mybir as mybir
from concourse.bass_utils import run_bass_kernel_spmd

N_CORES = 8
P_FULL = 50000
H = W = 28
M = H * W  # 784
KT = 50  # k-subtiles (of 128) per core
K_PER = KT * 128  # 6400
NT = KT // 2  # 25 double-tiles per plane
M2 = 2 * M  # 1568 bf16 cols per double-tile
NHALF = 392  # 784 / 2, per-PSUM-bank output chunk

BF16 = ml_dtypes.bfloat16
F32 = np.float32

B = 12  # double-tile buffers per plane (4 planes x B x 3136B/partition)
N_LANES = 12  # round-robin lanes on the SP HWDGE queue


def build_nc() -> bass.Bass:
    f32 = mybir.dt.float32
    bf16 = mybir.dt.bfloat16
    nc = bass.Bass("TRN2")
    pw_d = nc.dram_tensor("pw", [128, 2 * KT], bf16, kind="ExternalInput")
    planes_d = {
        name: nc.dram_tensor(name, [NT * 128, M2], bf16, kind="ExternalInput")
        for name in ("ihi", "ilo", "jhi", "jlo")
    }
    out_d = nc.dram_tensor("out", [6, M], f32, kind="ExternalOutput")

    tiled = {
        name: t[:, :].rearrange("(n p) m -> n p m", p=128)
        for name, t in planes_d.items()
    }

    from contextlib import ExitStack

    with ExitStack() as stk:
        pw_sb = stk.enter_context(nc.sbuf_tensor("pw_sb", [128, 2 * KT], bf16))
        streams = {
            name: stk.enter_context(
                nc.sbuf_tensor(f"s_{name}", [128, B * M2], bf16)
            )
            for name in ("ihi", "ilo", "jhi", "jlo")
        }
        o_ih = stk.enter_context(nc.sbuf_tensor("o_ih", [2, M], f32))
        o_il = stk.enter_context(nc.sbuf_tensor("o_il", [1, M], f32))
        o_jh = stk.enter_context(nc.sbuf_tensor("o_jh", [2, M], f32))
        o_jl = stk.enter_context(nc.sbuf_tensor("o_jl", [1, M], f32))
        ps = {
            ("i", "h"): tuple(
                stk.enter_context(nc.psum_tensor(f"ps_ih{c}", [2, NHALF], f32))
                for c in range(2)
            ),
            ("i", "l"): tuple(
                stk.enter_context(nc.psum_tensor(f"ps_il{c}", [1, NHALF], f32))
                for c in range(2)
            ),
            ("j", "h"): tuple(
                stk.enter_context(nc.psum_tensor(f"ps_jh{c}", [2, NHALF], f32))
                for c in range(2)
            ),
            ("j", "l"): tuple(
                stk.enter_context(nc.psum_tensor(f"ps_jl{c}", [1, NHALF], f32))
                for c in range(2)
            ),
        }
        sp_lanes = [
            stk.enter_context(nc.semaphore(f"spl{q}"))
            for q in range(N_LANES)
        ]
        pe_sem = stk.enter_context(nc.semaphore("pe_sem"))
        pe_i_sem = stk.enter_context(nc.semaphore("pe_i_sem"))
        dve_sem = stk.enter_context(nc.semaphore("dve_sem"))
        block = stk.enter_context(nc.Block(no_gpsimd_drain=True))

        outs = {("i", "h"): o_ih, ("i", "l"): o_il,
                ("j", "h"): o_jh, ("j", "l"): o_jl}

        def slot_cols(n):
            s = (n % B) * M2
            return slice(s, s + M2)

        # Per-queue round-robin lane bookkeeping (see module docstring).
        dma_records = {}

        def make_issuer(lanes):
            state = {"k": 0, "counts": [0] * len(lanes)}

            def issue(eng, dst, src, record_key):
                q = state["k"] % len(lanes)
                state["k"] += 1
                prev = state["counts"][q]
                ins = eng.dma_start(dst, src).then_inc(lanes[q], 16)
                if prev > 0:
                    ins.wait_op(lanes[q], 16 * prev, "sem-ge")
                state["counts"][q] = prev + 1
                dma_records.setdefault(record_key, []).append(
                    (lanes[q], 16 * (prev + 1))
                )

            return issue

        issue_sp = make_issuer(sp_lanes)

        @block.sync
        def _(sync):
            issue_sp(sync, pw_sb[:, :], pw_d[:, :], ("pw",))
            for n in range(NT):
                if n >= B:
                    # slot n%B was last used by double-tile n-B; wait until
                    # the PE consumed it (pe_sem counts finished double-tiles)
                    sync.wait_ge(pe_sem, n - B + 1)
                cols = slot_cols(n)
                for name in ("ihi", "ilo", "jhi", "jlo"):
                    issue_sp(
                        sync, streams[name][:, cols], tiled[name][n, :, :],
                        ("tile", n),
                    )
            # ship I's outputs as soon as the DVE evicted them (overlaps
            # J's last matmuls + eviction), then J's
            sync.wait_ge(dve_sem, 1)
            issue_sp(sync, out_d[0:2, :], o_ih[:, :], ("out",))
            issue_sp(sync, out_d[2:3, :], o_il[:, :], ("out",))
            sync.wait_ge(dve_sem, 2)
            issue_sp(sync, out_d[3:5, :], o_jh[:, :], ("out",))
            sync.wait_ge(dve_sem, 3)
            issue_sp(sync, out_d[5:6, :], o_jl[:, :], ("out",))
            for sem, v in dma_records[("out",)]:
                sync.wait_ge(sem, v)

        @block.tensor
        def _(tensor):
            for n in range(NT):
                if n == 0:
                    for sem, v in dma_records[("pw",)]:
                        tensor.wait_ge(sem, v)
                # records 0,1 = ihi/ilo DMAs; 2,3 = jhi/jlo. Waiting per
                # plane pair lets I's matmuls run while J's planes are
                # still in flight (on the last tile, I's eviction and
                # output DMA complete before the stream ends).
                recs = dma_records[("tile", n)]
                for sem, v in recs[:2]:
                    tensor.wait_ge(sem, v)
                cols = slot_cols(n)
                last = None

                def emit(mat, two, n=n, cols=cols):
                    s = 2 * n + two  # k-subtile index
                    start = s == 0
                    stop = s == KT - 1
                    w2 = pw_sb[:, 2 * s : 2 * s + 2]  # [128, 2] (p_hi, p_lo)
                    w1 = pw_sb[:, 2 * s : 2 * s + 1]  # [128, 1] (p_hi)
                    last = None
                    for c in range(2):
                        cs = slice(
                            cols.start + two * M + c * NHALF,
                            cols.start + two * M + (c + 1) * NHALF,
                        )
                        last = nc.tensor.matmul(
                            ps[(mat, "h")][c][:, :], w2,
                            streams[f"{mat}hi"][:, cs],
                            start=start, stop=stop,
                        )
                        last = nc.tensor.matmul(
                            ps[(mat, "l")][c][:, :], w1,
                            streams[f"{mat}lo"][:, cs],
                            start=start, stop=stop,
                        )
                    return last

                for two in range(2):
                    last = emit("i", two)
                if n == NT - 1:
                    last.then_inc(pe_i_sem, 1)
                for sem, v in recs[2:]:
                    tensor.wait_ge(sem, v)
                for two in range(2):
                    last = emit("j", two)
                last.then_inc(pe_sem, 1)

        @block.vector
        def _(vector):
            # I's PSUMs close one half-tile before J's (matrix-major order
            # on the last tile) -- evict + ship them while J still runs
            vector.wait_ge(pe_i_sem, 1)
            last = None
            for hl in ("h", "l"):
                for c in range(2):
                    cs = slice(c * NHALF, (c + 1) * NHALF)
                    last = nc.vector.tensor_copy(
                        outs[("i", hl)][:, cs], ps[("i", hl)][c][:, :]
                    )
            last.then_inc(dve_sem, 1)
            vector.wait_ge(pe_sem, NT)
            for hl in ("h", "l"):
                for c in range(2):
                    cs = slice(c * NHALF, (c + 1) * NHALF)
                    last = nc.vector.tensor_copy(
                        outs[("j", hl)][:, cs], ps[("j", hl)][c][:, :]
                    )
                # ship o_jh while o_jl is still being copied
                last.then_inc(dve_sem, 1)

    return nc


_NC_CACHE = None


def get_nc() -> bass.Bass:
    global _NC_CACHE
    if _NC_CACHE is None:
        _NC_CACHE = build_nc()
    return _NC_CACHE


def _split_hi_lo(a32: np.ndarray):
    hi = a32.astype(BF16)
    lo = (a32 - hi.astype(F32)).astype(BF16)
    return hi, lo


def _pack_pairs(plane: np.ndarray) -> np.ndarray:
    """[K_PER, M] -> [NT*128, 2*M]: subtiles 2n,2n+1 side by side so one
    DMA moves a fully contiguous [128 x 3136B] block."""
    return np.ascontiguousarray(
        plane.reshape(NT, 2, 128, M).transpose(0, 2, 1, 3).reshape(NT * 128, M2)
    )


def shard_inputs(p, I, J) -> list[dict]:
    p = np.asarray(p, dtype=F32)
    I = np.asarray(I, dtype=F32)
    J = np.asarray(J, dtype=F32)

    p_pad = np.zeros(N_CORES * K_PER, dtype=F32)
    p_pad[:P_FULL] = p

    in_maps = []
    for c in range(N_CORES):
        lo_k = c * K_PER
        hi_k = min(lo_k + K_PER, P_FULL)
        kc = hi_k - lo_k

        pc = p_pad[c * K_PER : (c + 1) * K_PER]
        phi, plo = _split_hi_lo(pc)
        pw = np.zeros((128, 2 * KT), dtype=BF16)
        pw[:, 0::2] = phi.reshape(KT, 128).T
        pw[:, 1::2] = plo.reshape(KT, 128).T

        im = {"pw": pw}
        for name, mat in (("i", I), ("j", J)):
            t = np.zeros((K_PER, M), dtype=F32)
            if kc > 0:
                t[:kc] = mat[:, lo_k:hi_k].T
            hi_p, lo_p = _split_hi_lo(t)
            im[f"{name}hi"] = _pack_pairs(hi_p)
            im[f"{name}lo"] = _pack_pairs(lo_p)
        in_maps.append(im)
    return in_maps


def run(p, I, J, inds1, inds2, trace=False, **run_kwargs):
    """Returns ((dgm1, dgm2), BassKernelResults)."""
    in_maps = shard_inputs(p, I, J)
    nc = get_nc()
    res = run_bass_kernel_spmd(
        nc, in_maps, list(range(N_CORES)), trace=trace, **run_kwargs
    )
    acc = np.zeros((6, M), dtype=np.float64)
    for r in res.results:
        acc += r["out"].astype(np.float64)
    Xp = (acc[0] + acc[1] + acc[2]).astype(F32).reshape(H, W)
    Yp = (acc[3] + acc[4] + acc[5]).astype(F32).reshape(H, W)
    inds1 = np.asarray(inds1)
    inds2 = np.asarray(inds2)
    dgm1 = Xp[inds1[:, 0], inds1[:, 1]].reshape(-1, 2)
    dgm2 = Yp[inds2[:, 0], inds2[:, 1]].reshape(-1, 2)
    return (dgm1, dgm2), res


def kernel(p, I, J, inds1, inds2):
    out, _ = run(p, I, J, inds1, inds2, trace=False)
    return out



# revision 7
# speedup vs baseline: 5.8801x; 5.8801x over previous
"""Distributed gathered-row matvec kernel for nn_CubicalModel_ISM.

Reference computes Xp = I @ p, Yp = J @ p (I, J: [784, 50000]) and then
gathers 100 (with repeats) elements from each 28x28 reshape. Only the
gathered rows matter, so the kernel:

  1. Host: dedupes the gather rows -> u1 (rows of I), u2 (rows of J),
     NR = |u1| + |u2| (~188 of the 1568 total rows). Builds
     A = concat(I[u1], J[u2]) : [NR, 50000] and computes only A @ p.
  2. Rounds A and p to bf16 (single plane). The bf16 quantization error
     of a 50k-term dot product concentrates around 3e-3 relative --
     far inside the 2e-2 gate -- while halving HBM traffic.
  3. Shards the contraction dim across 8 cores (6272 = 49*128 per core,
     zero padded). Per core a single DRAM stream [128, 49 + 49*NR] bf16
     carries the p chunk (first 49 cols) and the 49 transposed k-tiles
     of A, delivered by 8 chunked DMAs so the PE consumes tiles while
     later chunks are still in flight. 49 matmuls accumulate into one
     fp32 PSUM bank; the result is DMA'd straight from PSUM to DRAM.
  4. Host sums the 8 partial results (the "all-reduce"), then applies
     the inverse of the unique() mapping to emit the two [50, 2]
     diagrams.

Raw Bass (no Tile). Each DMA has its own semaphore (inc 16 on
completion); no DMA carries an embedded wait, standalone engine
wait_ge ops order everything else.
"""

import numpy as np
import ml_dtypes

import concourse.bass as bass
import concourse.mybir as mybir
from concourse.bass_utils import run_bass_kernel_spmd

N_CORES = 8
P_FULL = 50000
H = W = 28
M = H * W  # 784
KT = 49  # k-subtiles of 128 per core
K_PER = KT * 128  # 6272; 8 * 6272 = 50176 >= 50000
K_PAD = N_CORES * K_PER

# Tiles per chunk: last chunk is a single tile so the PE tail after the
# final chunk's semaphore fires is one 188-cycle matmul.
CHUNK_TILES = (7, 7, 7, 7, 7, 7, 6, 1)
assert sum(CHUNK_TILES) == KT

BF16 = ml_dtypes.bfloat16
F32 = np.float32


def build_nc(nr: int) -> bass.Bass:
    f32 = mybir.dt.float32
    bf16 = mybir.dt.bfloat16
    nc = bass.Bass("TRN2")
    ncols = KT + KT * nr  # p chunk cols, then 49 tiles of nr cols

    # Column ranges per chunk: chunk 0 also carries the p cols. Each
    # chunk gets its own DRAM tensor so the DMA source is fully
    # contiguous (a strided [128, x] read of one big tensor measured at
    # ~200 GB/s; contiguous blocks stream at full rate).
    bounds = []
    t0 = 0
    for g, gt in enumerate(CHUNK_TILES):
        c0 = 0 if g == 0 else KT + t0 * nr
        c1 = KT + (t0 + gt) * nr
        bounds.append((t0, t0 + gt, c0, c1))
        t0 += gt

    aw_ds = [
        nc.dram_tensor(f"aw{g}", [128, c1 - c0], bf16, kind="ExternalInput")
        for g, (_, _, c0, c1) in enumerate(bounds)
    ]
    out_d = nc.dram_tensor("outp", [1, nr], f32, kind="ExternalOutput")

    from contextlib import ExitStack

    with ExitStack() as stk:
        a_sb = stk.enter_context(nc.sbuf_tensor("a_sb", [128, ncols], bf16))
        o_sb = stk.enter_context(nc.sbuf_tensor("o_sb", [1, nr], f32))
        ps = stk.enter_context(nc.psum_tensor("ps", [1, nr], f32))
        ch_sems = [
            stk.enter_context(nc.semaphore(f"ch{g}"))
            for g in range(len(CHUNK_TILES))
        ]
        pe_sem = stk.enter_context(nc.semaphore("pe_sem"))
        dve_sem = stk.enter_context(nc.semaphore("dve_sem"))
        out_sem = stk.enter_context(nc.semaphore("out_sem"))
        block = stk.enter_context(nc.Block(no_gpsimd_drain=True))

        @block.sync
        def _(sync):
            for g, (_, _, c0, c1) in enumerate(bounds):
                sync.dma_start(a_sb[:, c0:c1], aw_ds[g][:, :]).then_inc(
                    ch_sems[g], 16
                )
            sync.wait_ge(dve_sem, 1)
            sync.dma_start(out_d[:, :], o_sb[:, :]).then_inc(out_sem, 16)
            sync.wait_ge(out_sem, 16)

        @block.tensor
        def _(tensor):
            last = None
            for g, (ta, tb, _, _) in enumerate(bounds):
                tensor.wait_ge(ch_sems[g], 16)
                for t in range(ta, tb):
                    last = nc.tensor.matmul(
                        ps[:, :],
                        a_sb[:, t : t + 1],
                        a_sb[:, KT + t * nr : KT + (t + 1) * nr],
                        start=(t == 0),
                        stop=(t == KT - 1),
                    )
            last.then_inc(pe_sem, 1)

        @block.vector
        def _(vector):
            vector.wait_ge(pe_sem, 1)
            nc.vector.tensor_copy(o_sb[:, :], ps[:, :]).then_inc(dve_sem, 1)

    return nc


_NC_CACHE: dict[int, bass.Bass] = {}


def get_nc(nr: int) -> bass.Bass:
    if nr not in _NC_CACHE:
        _NC_CACHE[nr] = build_nc(nr)
    return _NC_CACHE[nr]


def shard_inputs(A: np.ndarray, p: np.ndarray) -> list[dict]:
    """A: [NR, 50000] f32, p: [50000] f32 -> 8 per-core input maps."""
    nr = A.shape[0]
    Ab = np.zeros((nr, K_PAD), dtype=BF16)
    Ab[:, :P_FULL] = A.astype(BF16)
    pb = np.zeros(K_PAD, dtype=BF16)
    pb[:P_FULL] = p.astype(BF16)

    bounds = []
    t0 = 0
    for gt in CHUNK_TILES:
        bounds.append((t0, t0 + gt))
        t0 += gt

    in_maps = []
    for c in range(N_CORES):
        k0 = c * K_PER
        pw = np.ascontiguousarray(pb[k0 : k0 + K_PER].reshape(KT, 128).T)
        tiles = (
            Ab[:, k0 : k0 + K_PER]
            .T.reshape(KT, 128, nr)
            .transpose(1, 0, 2)
            .reshape(128, KT * nr)
        )
        im = {}
        for g, (ta, tb) in enumerate(bounds):
            part = tiles[:, ta * nr : tb * nr]
            if g == 0:
                part = np.concatenate([pw, part], axis=1)
            im[f"aw{g}"] = np.ascontiguousarray(part)
        in_maps.append(im)
    return in_maps


def run(p, I, J, inds1, inds2, trace=False, **run_kwargs):
    """Returns ((dgm1, dgm2), BassKernelResults)."""
    p = np.asarray(p, dtype=F32)
    I = np.asarray(I, dtype=F32)
    J = np.asarray(J, dtype=F32)
    inds1 = np.asarray(inds1)
    inds2 = np.asarray(inds2)

    rows1 = inds1[:, 0].astype(np.int64) * W + inds1[:, 1].astype(np.int64)
    rows2 = inds2[:, 0].astype(np.int64) * W + inds2[:, 1].astype(np.int64)
    u1, inv1 = np.unique(rows1, return_inverse=True)
    u2, inv2 = np.unique(rows2, return_inverse=True)
    n1 = len(u1)

    A = np.concatenate([I[u1], J[u2]], axis=0)
    nr = A.shape[0]

    in_maps = shard_inputs(A, p)
    nc = get_nc(nr)
    res = run_bass_kernel_spmd(
        nc, in_maps, list(range(N_CORES)), trace=trace, **run_kwargs
    )
    tot = np.zeros(nr, dtype=np.float64)
    for r in res.results:
        tot += r["outp"][0].astype(np.float64)
    dgm1 = tot[:n1][inv1].reshape(-1, 2).astype(F32)
    dgm2 = tot[n1:][inv2].reshape(-1, 2).astype(F32)
    return (dgm1, dgm2), res


def kernel(p, I, J, inds1, inds2):
    out, _ = run(p, I, J, inds1, inds2, trace=False)
    return out
